# revision 23
# baseline (speedup 1.0000x reference)
"""Additive (Bahdanau) attention kernel for Trainium2, 8 NeuronCores.

Problem (full shapes): query (32,256) f32, values (32,8192,256) f32,
W1 (256,128), W2 (256,128), V (128,1).
  proj_v = values @ W1                      (B,S,U)
  proj_q = query @ W2                       (B,1,U)
  score  = tanh(proj_v + proj_q) @ V        (B,S,1)
  attn   = softmax(score, axis=1)           (B,S,1)
  context= sum(attn * values, axis=1)       (B,D)
Returns (context, attn).

Sharding: data-parallel over batch, 4 batches per core, weights replicated.
Single pass over `values` (the 268MB tensor): scores and the weighted sum
are computed in the same streaming loop; softmax normalization is applied
at the end of each batch (no max-subtraction needed: |score| <= ||V||_1 ~ 5).
"""

import sys

sys.path.insert(0, "/opt/trn_rl_repo")

import numpy as np

import concourse.bass as bass
import concourse.tile as tile
from concourse import bacc, mybir
from concourse.bass_utils import run_bass_kernel_spmd
from concourse.masks import make_identity

B, S, D, Q, U = 32, 8192, 256, 256, 128
NCORES = 8
BL = B // NCORES  # batches per core

F32 = mybir.dt.float32
BF16 = mybir.dt.bfloat16

# group = GT tiles of 128 seq rows processed together
GT = 4
GROUP_ROWS = 128 * GT  # 512
NGROUPS = S // GROUP_ROWS  # 16

_CACHE = {}

import os

STAGE = int(os.environ.get("KSTAGE", "9"))  # 9 = full kernel


def _build():
    nc = bacc.Bacc()

    values_ext = nc.declare_dram_parameter("values", [BL, S, D], F32, isOutput=False)
    query_ext = nc.declare_dram_parameter("query", [BL, Q], F32, isOutput=False)
    w1_ext = nc.declare_dram_parameter("W1", [D, U], F32, isOutput=False)
    w2_ext = nc.declare_dram_parameter("W2", [Q, U], F32, isOutput=False)
    v_ext = nc.declare_dram_parameter("V", [U, 1], F32, isOutput=False)
    ctx_ext = nc.declare_dram_parameter("context", [BL, D], F32, isOutput=True)
    wout_ext = nc.declare_dram_parameter("weights", [BL, S], F32, isOutput=True)

    KO = D // 128  # 2 contraction chunks for D=256

    with tile.TileContext(nc) as tc:
        with (
            tc.tile_pool(name="consts", bufs=1) as consts,
            tc.tile_pool(name="vload", bufs=3) as vload,
            tc.tile_pool(name="vbf", bufs=3) as vbfp,
            tc.tile_pool(name="vtp", bufs=2) as vtp,
            tc.tile_pool(name="work", bufs=2) as work,
            tc.tile_pool(name="wbatch", bufs=2) as wbp,
            tc.tile_pool(name="epi", bufs=2) as epi,
            tc.tile_pool(name="ps_vt", bufs=3, space="PSUM") as ps_vt,
            tc.tile_pool(name="ps_proj", bufs=2, space="PSUM") as ps_proj,
            tc.tile_pool(name="ps_w", bufs=1, space="PSUM") as ps_w,
            tc.tile_pool(name="ps_ctx", bufs=1, space="PSUM") as ps_ctx,
            tc.tile_pool(name="ps_epi", bufs=1, space="PSUM") as ps_epi,
        ):
            # ---- constants ----
            ident = consts.tile([128, 128], BF16)
            make_identity(nc, ident)

            # W1: (D,U) -> sbuf [128, KO, U], chunk ko holds rows ko*128..+128
            w1_f32 = consts.tile([128, KO, U], F32)
            nc.sync.dma_start(w1_f32, w1_ext.rearrange("(ko p) u -> p ko u", p=128))
            w1_bf = consts.tile([128, KO, U], BF16)
            nc.vector.tensor_copy(w1_bf, w1_f32)

            w2_f32 = consts.tile([128, KO, U], F32)
            nc.sync.dma_start(w2_f32, w2_ext.rearrange("(ko p) u -> p ko u", p=128))
            w2_bf = consts.tile([128, KO, U], BF16)
            nc.vector.tensor_copy(w2_bf, w2_f32)

            v_f32 = consts.tile([128, 1], F32)
            nc.sync.dma_start(v_f32, v_ext[:, :])
            v_bf = consts.tile([128, 1], BF16)
            nc.vector.tensor_copy(v_bf, v_f32)

            ones_col = consts.tile([128, 1], F32)
            nc.gpsimd.memset(ones_col, 1.0)
            ones_row = consts.tile([1, 128], F32)
            nc.gpsimd.memset(ones_row, 1.0)

            # query: (BL, Q) f32 -> bf16, transpose to qT [128, KO, BL]
            q_f32 = consts.tile([BL, Q], F32)
            nc.sync.dma_start(q_f32, query_ext[:, :])
            q_bf = consts.tile([BL, Q], BF16)
            nc.vector.tensor_copy(q_bf, q_f32)
            psum_qt = ps_epi.tile([128, KO * BL], BF16, tag="epi")
            for ko in range(KO):
                nc.tensor.transpose(
                    psum_qt[:, ko * BL : (ko + 1) * BL],
                    q_bf[:, ko * 128 : (ko + 1) * 128],
                    ident[:BL, :BL],
                )
            qt_bf = consts.tile([128, KO, BL], BF16)
            nc.vector.tensor_copy(qt_bf, psum_qt.rearrange("p (ko b) -> p ko b", ko=KO))

            # proj_q = W2^T q : psum (U=128, BL)
            psum_pq = ps_epi.tile([128, BL], F32, tag="epi")
            for ko in range(KO):
                nc.tensor.matmul(
                    psum_pq,
                    w2_bf[:, ko],
                    qt_bf[:, ko],
                    start=(ko == 0),
                    stop=(ko == KO - 1),
                )
            pq_f32 = consts.tile([128, BL], F32)
            nc.vector.tensor_copy(pq_f32, psum_pq)

            if STAGE < 9 or STAGE == 80:
                zt = consts.tile([128, 256], F32)
                nc.gpsimd.memset(zt, 0.0)
                for b in range(BL):
                    nc.sync.dma_start(ctx_ext[b : b + 1, :], zt[0:1, :])
                    nc.sync.dma_start(
                        wout_ext[b].rearrange("(c x) -> c x", x=128), zt[:64, :128]
                    )
            # ---- main loop ----
            for b in range(BL):
                w_batch = wbp.tile([128, S // 128], BF16)  # exp(score) columns
                psum_ctx = ps_ctx.tile([1, D], F32)

                for g in range(NGROUPS):
                    # load 512 seq rows: [128, GT, D] f32
                    v_f = vload.tile([128, GT, D], F32)
                    nc.sync.dma_start(
                        v_f,
                        values_ext[
                            b, g * GROUP_ROWS : (g + 1) * GROUP_ROWS, :
                        ].rearrange("(t p) d -> p t d", p=128),
                    )
                    v_b = vbfp.tile([128, GT, D], BF16)
                    nc.gpsimd.tensor_copy(v_b, v_f)
                    if STAGE < 2:
                        continue

                    # transpose to vT chunks via REGULAR matmul with identity
                    # moving operand (counts as PE-busy for HAM, unlike
                    # transpose-mode): out = v_chunk.T @ I = vT chunk (f32)
                    vt_b = vtp.tile([128, KO, GROUP_ROWS], BF16)
                    for ko in range(KO):
                        psum_vt = ps_vt.tile([128, GROUP_ROWS], F32, tag="vt")
                        for t in range(GT):
                            nc.tensor.matmul(
                                psum_vt[:, t * 128 : (t + 1) * 128],
                                v_b[:, t, ko * 128 : (ko + 1) * 128],
                                ident,
                                start=True,
                                stop=True,
                            )
                        nc.vector.tensor_copy(vt_b[:, ko], psum_vt)
                    if STAGE < 3:
                        continue

                    # proj^T = W1^T vT : psum (U=128, GROUP_ROWS) f32
                    psum_proj = ps_proj.tile([128, GROUP_ROWS], F32)
                    for ko in range(KO):
                        nc.tensor.matmul(
                            psum_proj,
                            w1_bf[:, ko],
                            vt_b[:, ko],
                            start=(ko == 0),
                            stop=(ko == KO - 1),
                        )

                    # tanh(proj + pq[b]) -> sbuf bf16 (U, GROUP_ROWS)
                    th_b = work.tile([128, GROUP_ROWS], BF16, tag="tanh")
                    nc.scalar.activation(
                        th_b,
                        psum_proj,
                        mybir.ActivationFunctionType.Tanh,
                        bias=pq_f32[:, b : b + 1],
                    )
                    if STAGE < 4:
                        continue

                    # score columns: for each 128-row tile, (S_t,1) = th^T V
                    psum_s = ps_w.tile([128, GT], F32)
                    for t in range(GT):
                        nc.tensor.matmul(
                            psum_s[:, t : t + 1],
                            th_b[:, t * 128 : (t + 1) * 128],
                            v_bf,
                            start=True,
                            stop=True,
                        )

                    # exp -> w_batch slice (bf16)
                    nc.scalar.activation(
                        w_batch[:, g * GT : (g + 1) * GT],
                        psum_s,
                        mybir.ActivationFunctionType.Exp,
                    )
                    if STAGE < 5:
                        continue

                    # phase 2: context += w^T v  (accumulate over whole batch)
                    for t in range(GT):
                        i = g * GT + t
                        nc.tensor.matmul(
                            psum_ctx,
                            w_batch[:, i : i + 1],
                            v_b[:, t],
                            start=(i == 0),
                            stop=(i == S // 128 - 1),
                        )

                if STAGE < 6:
                    continue
                # ---- batch epilogue ----
                # l = sum of w
                lsum = epi.tile([128, 1], F32, tag="lsum")
                nc.vector.reduce_sum(lsum, w_batch, axis=mybir.AxisListType.X)
                psum_l = ps_epi.tile([1, 1], F32, tag="epi")
                nc.tensor.matmul(psum_l, ones_col, lsum, start=True, stop=True)
                rl = epi.tile([1, 1], F32, tag="rl")
                nc.vector.reciprocal(rl, psum_l)
                if STAGE < 7:
                    continue
                # broadcast 1/l to 128 partitions
                psum_rb = ps_epi.tile([128, 1], F32, tag="epi")
                nc.tensor.matmul(psum_rb, ones_row, rl, start=True, stop=True)
                rb = epi.tile([128, 1], F32, tag="rb")
                nc.vector.tensor_copy(rb, psum_rb)
                if STAGE < 8:
                    continue

                # context out
                ctx_sb = epi.tile([1, D], F32, tag="ctx")
                nc.scalar.activation(
                    ctx_sb,
                    psum_ctx,
                    mybir.ActivationFunctionType.Identity,
                    scale=(1.0 if STAGE == 80 else rl),
                )
                nc.sync.dma_start(ctx_ext[b : b + 1, :], ctx_sb)
                if STAGE < 9 or STAGE == 80:
                    continue

                # weights out: transpose w_batch (128, 64) -> (64, 128), scale
                psum_wt = ps_proj.tile([S // 128, 128], BF16, tag="psum_proj")
                nc.tensor.transpose(psum_wt, w_batch, ident)
                wn = epi.tile([S // 128, 128], F32, tag="wn")
                nc.scalar.activation(
                    wn,
                    psum_wt,
                    mybir.ActivationFunctionType.Identity,
                    scale=rb[: S // 128, :],
                )
                nc.sync.dma_start(
                    wout_ext[b].rearrange("(c x) -> c x", x=128), wn
                )

    nc.finalize()
    return nc


def _get_nc():
    if "nc" not in _CACHE:
        _CACHE["nc"] = _build()
    return _CACHE["nc"]


def kernel(query, values, W1, W2, V):
    query = np.ascontiguousarray(np.asarray(query, dtype=np.float32))
    values = np.ascontiguousarray(np.asarray(values, dtype=np.float32))
    W1 = np.ascontiguousarray(np.asarray(W1, dtype=np.float32))
    W2 = np.ascontiguousarray(np.asarray(W2, dtype=np.float32))
    V = np.ascontiguousarray(np.asarray(V, dtype=np.float32))

    nc = _get_nc()
    in_maps = [
        {
            "values": values[c * BL : (c + 1) * BL],
            "query": query[c * BL : (c + 1) * BL],
            "W1": W1,
            "W2": W2,
            "V": V,
        }
        for c in range(NCORES)
    ]
    res = run_bass_kernel_spmd(nc, in_maps, core_ids=list(range(NCORES)))
    _CACHE["last_result"] = res
    context = np.concatenate([r["context"] for r in res.results], axis=0)
    weights = np.concatenate([r["weights"] for r in res.results], axis=0)
    return context, weights.reshape(B, S, 1)


# revision 25
# speedup vs baseline: 2.6084x; 2.6084x over previous
"""Additive (Bahdanau) attention kernel for Trainium2, 8 NeuronCores.

Problem (full shapes): query (32,256) f32, values (32,8192,256) f32,
W1 (256,128), W2 (256,128), V (128,1).
  proj_v = values @ W1                      (B,S,U)
  proj_q = query @ W2                       (B,1,U)
  score  = tanh(proj_v + proj_q) @ V        (B,S,1)
  attn   = softmax(score, axis=1)           (B,S,1)
  context= sum(attn * values, axis=1)       (B,D)
Returns (context, attn).

Sharding: data-parallel over batch, 4 batches per core, weights replicated.
Single pass over `values` (the 268MB tensor): scores and the weighted sum
are computed in the same streaming loop; softmax normalization is applied
at the end of each batch (no max-subtraction needed: |score| <= ||V||_1 ~ 5).
"""

import sys

sys.path.insert(0, "/opt/trn_rl_repo")

import numpy as np

import concourse.bass as bass
import concourse.tile as tile
from concourse import bacc, mybir
from concourse.bass_utils import run_bass_kernel_spmd
from concourse.masks import make_identity

B, S, D, Q, U = 32, 8192, 256, 256, 128
NCORES = 8
BL = B // NCORES  # batches per core

F32 = mybir.dt.float32
BF16 = mybir.dt.bfloat16

# group = GT tiles of 128 seq rows processed together
GT = 4
GROUP_ROWS = 128 * GT  # 512
NGROUPS = S // GROUP_ROWS  # 16

_CACHE = {}

import os

STAGE = int(os.environ.get("KSTAGE", "9"))  # 9 = full kernel


def _build():
    nc = bacc.Bacc()

    values_ext = nc.declare_dram_parameter("values", [BL, S, D], F32, isOutput=False)
    query_ext = nc.declare_dram_parameter("query", [BL, Q], F32, isOutput=False)
    w1_ext = nc.declare_dram_parameter("W1", [D, U], F32, isOutput=False)
    w2_ext = nc.declare_dram_parameter("W2", [Q, U], F32, isOutput=False)
    v_ext = nc.declare_dram_parameter("V", [U, 1], F32, isOutput=False)
    ctx_ext = nc.declare_dram_parameter("context", [BL, D], F32, isOutput=True)
    wout_ext = nc.declare_dram_parameter("weights", [BL, S], F32, isOutput=True)

    KO = D // 128  # 2 contraction chunks for D=256

    with tile.TileContext(nc) as tc:
        with (
            tc.tile_pool(name="consts", bufs=1) as consts,
            tc.tile_pool(name="vload", bufs=5) as vload,
            tc.tile_pool(name="vbf", bufs=4) as vbfp,
            tc.tile_pool(name="vtp", bufs=3) as vtp,
            tc.tile_pool(name="work", bufs=3) as work,
            tc.tile_pool(name="wbatch", bufs=2) as wbp,
            tc.tile_pool(name="epi", bufs=2) as epi,
            tc.tile_pool(name="ps_vt", bufs=3, space="PSUM") as ps_vt,
            tc.tile_pool(name="ps_proj", bufs=2, space="PSUM") as ps_proj,
            tc.tile_pool(name="ps_w", bufs=1, space="PSUM") as ps_w,
            tc.tile_pool(name="ps_ctx", bufs=1, space="PSUM") as ps_ctx,
            tc.tile_pool(name="ps_epi", bufs=1, space="PSUM") as ps_epi,
        ):
            # ---- constants ----
            ident = consts.tile([128, 128], BF16)
            make_identity(nc, ident)

            # W1: (D,U) -> sbuf [128, KO, U], chunk ko holds rows ko*128..+128
            w1_f32 = consts.tile([128, KO, U], F32)
            nc.sync.dma_start(w1_f32, w1_ext.rearrange("(ko p) u -> p ko u", p=128))
            w1_bf = consts.tile([128, KO, U], BF16)
            nc.vector.tensor_copy(w1_bf, w1_f32)

            w2_f32 = consts.tile([128, KO, U], F32)
            nc.sync.dma_start(w2_f32, w2_ext.rearrange("(ko p) u -> p ko u", p=128))
            w2_bf = consts.tile([128, KO, U], BF16)
            nc.vector.tensor_copy(w2_bf, w2_f32)

            v_f32 = consts.tile([128, 1], F32)
            nc.sync.dma_start(v_f32, v_ext[:, :])
            v_bf = consts.tile([128, 1], BF16)
            nc.vector.tensor_copy(v_bf, v_f32)

            ones_col = consts.tile([128, 1], F32)
            nc.gpsimd.memset(ones_col, 1.0)
            ones_row = consts.tile([1, 128], F32)
            nc.gpsimd.memset(ones_row, 1.0)

            # query: (BL, Q) f32 -> bf16, transpose to qT [128, KO, BL]
            q_f32 = consts.tile([BL, Q], F32)
            nc.sync.dma_start(q_f32, query_ext[:, :])
            q_bf = consts.tile([BL, Q], BF16)
            nc.vector.tensor_copy(q_bf, q_f32)
            psum_qt = ps_epi.tile([128, KO * BL], BF16, tag="epi")
            for ko in range(KO):
                nc.tensor.transpose(
                    psum_qt[:, ko * BL : (ko + 1) * BL],
                    q_bf[:, ko * 128 : (ko + 1) * 128],
                    ident[:BL, :BL],
                )
            qt_bf = consts.tile([128, KO, BL], BF16)
            nc.vector.tensor_copy(qt_bf, psum_qt.rearrange("p (ko b) -> p ko b", ko=KO))

            # proj_q = W2^T q : psum (U=128, BL)
            psum_pq = ps_epi.tile([128, BL], F32, tag="epi")
            for ko in range(KO):
                nc.tensor.matmul(
                    psum_pq,
                    w2_bf[:, ko],
                    qt_bf[:, ko],
                    start=(ko == 0),
                    stop=(ko == KO - 1),
                )
            pq_f32 = consts.tile([128, BL], F32)
            nc.vector.tensor_copy(pq_f32, psum_pq)

            if STAGE < 9 or STAGE == 80:
                zt = consts.tile([128, 256], F32)
                nc.gpsimd.memset(zt, 0.0)
                for b in range(BL):
                    nc.sync.dma_start(ctx_ext[b : b + 1, :], zt[0:1, :])
                    nc.sync.dma_start(
                        wout_ext[b].rearrange("(c x) -> c x", x=128), zt[:64, :128]
                    )
            # ---- main loop ----
            for b in range(BL):
                w_batch = wbp.tile([128, S // 128], BF16)  # exp(score) columns
                psum_ctx = ps_ctx.tile([1, D], F32)

                for g in range(NGROUPS):
                    # load 512 seq rows: [128, GT, D] f32
                    v_f = vload.tile([128, GT, D], F32)
                    nc.sync.dma_start(
                        v_f,
                        values_ext[
                            b, g * GROUP_ROWS : (g + 1) * GROUP_ROWS, :
                        ].rearrange("(t p) d -> p t d", p=128),
                    )
                    v_b = vbfp.tile([128, GT, D], BF16)
                    nc.vector.tensor_copy(v_b, v_f)
                    if STAGE < 2:
                        continue

                    # transpose to vT chunks: psum [128, KO, GROUP_ROWS] bf16
                    psum_vt = ps_vt.tile([128, KO, GROUP_ROWS], BF16)
                    for t in range(GT):
                        for ko in range(KO):
                            nc.tensor.transpose(
                                psum_vt[:, ko, t * 128 : (t + 1) * 128],
                                v_b[:, t, ko * 128 : (ko + 1) * 128],
                                ident,
                            )
                    vt_b = vtp.tile([128, KO, GROUP_ROWS], BF16)
                    nc.vector.tensor_copy(vt_b, psum_vt)
                    if STAGE < 3:
                        continue

                    # proj^T = W1^T vT : psum (U=128, GROUP_ROWS) f32
                    psum_proj = ps_proj.tile([128, GROUP_ROWS], F32)
                    for ko in range(KO):
                        nc.tensor.matmul(
                            psum_proj,
                            w1_bf[:, ko],
                            vt_b[:, ko],
                            start=(ko == 0),
                            stop=(ko == KO - 1),
                        )

                    # tanh(proj + pq[b]) -> sbuf bf16 (U, GROUP_ROWS)
                    th_b = work.tile([128, GROUP_ROWS], BF16, tag="tanh")
                    nc.scalar.activation(
                        th_b,
                        psum_proj,
                        mybir.ActivationFunctionType.Tanh,
                        bias=pq_f32[:, b : b + 1],
                    )
                    if STAGE < 4:
                        continue

                    # score columns: for each 128-row tile, (S_t,1) = th^T V
                    psum_s = ps_w.tile([128, GT], F32)
                    for t in range(GT):
                        nc.tensor.matmul(
                            psum_s[:, t : t + 1],
                            th_b[:, t * 128 : (t + 1) * 128],
                            v_bf,
                            start=True,
                            stop=True,
                        )

                    # exp -> w_batch slice (bf16)
                    nc.scalar.activation(
                        w_batch[:, g * GT : (g + 1) * GT],
                        psum_s,
                        mybir.ActivationFunctionType.Exp,
                    )
                    if STAGE < 5:
                        continue

                    # phase 2: context += w^T v  (accumulate over whole batch)
                    for t in range(GT):
                        i = g * GT + t
                        nc.tensor.matmul(
                            psum_ctx,
                            w_batch[:, i : i + 1],
                            v_b[:, t],
                            start=(i == 0),
                            stop=(i == S // 128 - 1),
                        )

                if STAGE < 6:
                    continue
                # ---- batch epilogue ----
                # l = sum of w
                lsum = epi.tile([128, 1], F32, tag="lsum")
                nc.vector.reduce_sum(lsum, w_batch, axis=mybir.AxisListType.X)
                psum_l = ps_epi.tile([1, 1], F32, tag="epi")
                nc.tensor.matmul(psum_l, ones_col, lsum, start=True, stop=True)
                rl = epi.tile([1, 1], F32, tag="rl")
                nc.vector.reciprocal(rl, psum_l)
                if STAGE < 7:
                    continue
                # broadcast 1/l to 128 partitions
                psum_rb = ps_epi.tile([128, 1], F32, tag="epi")
                nc.tensor.matmul(psum_rb, ones_row, rl, start=True, stop=True)
                rb = epi.tile([128, 1], F32, tag="rb")
                nc.vector.tensor_copy(rb, psum_rb)
                if STAGE < 8:
                    continue

                # context out
                ctx_sb = epi.tile([1, D], F32, tag="ctx")
                nc.scalar.activation(
                    ctx_sb,
                    psum_ctx,
                    mybir.ActivationFunctionType.Identity,
                    scale=(1.0 if STAGE == 80 else rl),
                )
                nc.sync.dma_start(ctx_ext[b : b + 1, :], ctx_sb)
                if STAGE < 9 or STAGE == 80:
                    continue

                # weights out: transpose w_batch (128, 64) -> (64, 128), scale
                psum_wt = ps_proj.tile([S // 128, 128], BF16, tag="psum_proj")
                nc.tensor.transpose(psum_wt, w_batch, ident)
                wn = epi.tile([S // 128, 128], F32, tag="wn")
                nc.scalar.activation(
                    wn,
                    psum_wt,
                    mybir.ActivationFunctionType.Identity,
                    scale=rb[: S // 128, :],
                )
                nc.sync.dma_start(
                    wout_ext[b].rearrange("(c x) -> c x", x=128), wn
                )

    nc.finalize()
    return nc


def _get_nc():
    if "nc" not in _CACHE:
        _CACHE["nc"] = _build()
    return _CACHE["nc"]


def kernel(query, values, W1, W2, V):
    query = np.ascontiguousarray(np.asarray(query, dtype=np.float32))
    values = np.ascontiguousarray(np.asarray(values, dtype=np.float32))
    W1 = np.ascontiguousarray(np.asarray(W1, dtype=np.float32))
    W2 = np.ascontiguousarray(np.asarray(W2, dtype=np.float32))
    V = np.ascontiguousarray(np.asarray(V, dtype=np.float32))

    nc = _get_nc()
    in_maps = [
        {
            "values": values[c * BL : (c + 1) * BL],
            "query": query[c * BL : (c + 1) * BL],
            "W1": W1,
            "W2": W2,
            "V": V,
        }
        for c in range(NCORES)
    ]
    res = run_bass_kernel_spmd(nc, in_maps, core_ids=list(range(NCORES)))
    _CACHE["last_result"] = res
    context = np.concatenate([r["context"] for r in res.results], axis=0)
    weights = np.concatenate([r["weights"] for r in res.results], axis=0)
    return context, weights.reshape(B, S, 1)
